# revision 9
# baseline (speedup 1.0000x reference)
"""Controlled neural-SDE Euler-Maruyama kernel for 8 Trainium2 NeuronCores.

Strategy (data-parallel over batch, per sharding hint):
  - batch 4096 -> 8 cores x 512. Each core runs the full 199-step time loop.
  - Feature-major layout on chip: activations are (features, batch) so every
    FFN layer is matmul(out_psum, lhsT=W_block, rhs=act) with the batch (512)
    as the moving/free dimension. Weights are packed once into one SBUF tile.
  - Matmuls run in float32r (TF32-class, 1 cycle/row at free dim 512 vs 4 for
    fp32). The x state is kept in exact fp32 in a separate tile; a rounded
    float32r copy feeds the matmuls.
  - The time/bias terms are folded into per-step L1 bias tables computed on
    the host:
      * alpha input concat([t, x]) == matmul over x + per-step bias
        t_s * Wa1[0,:].
      * drift L3 bias b_mu is folded into the trajectory reparametrization
        x'_s = x_s - t_s * b_mu; all four L1 layers get the per-step bias
        correction W1x.T @ (t_s * b_mu), and the host adds t_s * b_mu back to
        the x output. (With the reference's zero-init biases this is a no-op.)
  - Per step: 32 matmuls, 10 scalar-engine activations (bias+relu psum->sbuf),
    12 vector-engine ops (bias+relu copies for drift/diffusion, then the
    fused update u=(mu*dt+x), w=(sig+b)*dW, x_new=u+w, and the f32r copy).
  - rewards row, action tile and x tile are streamed out per step via HWDGE.
"""
import os
import sys

sys.path.insert(0, "/opt/trn_rl_repo")

import numpy as np
from contextlib import ExitStack

import concourse.bass as bass
import concourse.bacc as bacc
import concourse.mybir as mybir
from concourse import tile
from concourse.bass_utils import run_bass_kernel_spmd

# Optional on-disk NEFF cache for the dev loop (harmless if unset).
if os.environ.get("KSDE_NEFF_CACHE"):
    import hashlib
    import shutil
    import concourse.bass2jax as _b2j

    _CACHE_DIR = os.environ["KSDE_NEFF_CACHE"]
    os.makedirs(_CACHE_DIR, exist_ok=True)
    _orig_compile = _b2j.compile_bir_kernel

    def _cached_compile(bir_json, tmpdir, neff_name="file.neff"):
        key = hashlib.sha256(bir_json).hexdigest()
        cached = os.path.join(_CACHE_DIR, key + ".neff")
        if os.path.exists(cached):
            dst = os.path.join(tmpdir, neff_name)
            shutil.copy(cached, dst)
            return dst
        out = _orig_compile(bir_json, tmpdir, neff_name)
        shutil.copy(out, cached)
        return out

    _b2j.compile_bir_kernel = _cached_compile

B, L, D = 4096, 200, 64
HID = 256
NCORES = 8
BC = B // NCORES
NSTEP = int(os.environ.get("KSDE_NSTEP", L - 1))

f32 = mybir.dt.float32
f32r = mybir.dt.float32r
AF = mybir.ActivationFunctionType
ALU = mybir.AluOpType

# ---------------------------------------------------------------- layouts --

_MATS = {
    # name: (K_rows, M_cols)   (lhsT layout: W[k, m] with k on partitions)
    "A1": (D, HID), "A2": (HID, HID), "A3": (HID, D),
    "R1": (2 * D, HID), "R2": (HID, HID), "R3": (HID, 1),
    "D1": (2 * D, HID), "D2": (HID, HID), "D3": (HID, D),
    "S1": (D, HID), "S2": (HID, HID), "S3": (HID, D),
    "F1": (D, HID), "F2": (HID, HID), "F3": (HID, 1),
}


def _build_wmeta():
    """Column layout of the packed weights tile: wmeta[name][mt][kt] ->
    (col, width, rows)."""
    wmeta = {}
    col = 0
    for name, (K, M) in _MATS.items():
        blocks_m = []
        for m0 in range(0, M, 128):
            width = min(128, M - m0)
            blocks_k = []
            for k0 in range(0, K, 128):
                rows = min(128, K - k0)
                blocks_k.append((col, width, rows, k0, m0))
                col += width
            blocks_m.append(blocks_k)
        wmeta[name] = blocks_m
    return wmeta, col


_WMETA, NW = _build_wmeta()

# constants tile columns
_CB = {}
_cb_col = 0
for _name, _width in [
    ("ba1", 2 * NSTEP), ("br1", 2 * NSTEP), ("bd1", 2 * NSTEP),
    ("bs1", 2 * NSTEP),
    ("ba2", 2), ("ba3", 1), ("br2", 2), ("br3", 1), ("bd2", 2),
    ("bs2", 2), ("bs3", 1), ("bf1", 2), ("bf2", 2), ("bf3", 1),
]:
    _CB[_name] = _cb_col
    _cb_col += _width
NB = _cb_col

# ---------------------------------------------------------------- program --

_PROG_CACHE = {}


def _build_program(dts):
    """Build (and schedule) the per-core Bass program. dts: python floats of
    the 199 fp32 time deltas (compile-time immediates)."""
    nc = bacc.Bacc("TRN2", target_bir_lowering=False, debug=False)

    x0r_in = nc.declare_dram_parameter("x0r", [D, BC], f32r, isOutput=False)
    x0f_in = nc.declare_dram_parameter("x0f", [D, BC], f32, isOutput=False)
    dw_in = nc.declare_dram_parameter("dw", [NSTEP, D, BC], f32, isOutput=False)
    wr_in = nc.declare_dram_parameter("wr", [128, NW], f32r, isOutput=False)
    cb_in = nc.declare_dram_parameter("cb", [128, NB], f32, isOutput=False)

    xs_out = nc.declare_dram_parameter("xs", [NSTEP, D, BC], f32, isOutput=True)
    acts_out = nc.declare_dram_parameter("acts", [NSTEP, D, BC], f32r, isOutput=True)
    rews_out = nc.declare_dram_parameter("rews", [NSTEP + 1, BC], f32, isOutput=True)

    with ExitStack() as ctx:
        tc = ctx.enter_context(tile.TileContext(nc))
        wp = ctx.enter_context(tc.tile_pool(name="wp", bufs=1))
        xap = ctx.enter_context(tc.tile_pool(name="xap", bufs=3))
        xfp = ctx.enter_context(tc.tile_pool(name="xfp", bufs=3))
        dwp = ctx.enter_context(tc.tile_pool(name="dwp", bufs=4))
        hp = ctx.enter_context(tc.tile_pool(name="hp", bufs=2))
        uwp = ctx.enter_context(tc.tile_pool(name="uwp", bufs=2))
        rp = ctx.enter_context(tc.tile_pool(name="rp", bufs=2))
        psp = ctx.enter_context(tc.tile_pool(name="psp", bufs=8, space="PSUM"))

        wrt = wp.tile([128, NW], f32r)
        cbt = wp.tile([128, NB], f32)
        nc.sync.dma_start(wrt[:], wr_in[:])
        nc.sync.dma_start(cbt[:], cb_in[:])

        def wap(name, mt, kt):
            col, width, rows, _, _ = _WMETA[name][mt][kt]
            return wrt[0:rows, col:col + width]

        def bias(name, k, parts=128, step=None):
            col = _CB[name] + (k * NSTEP + step if step is not None else k)
            return cbt[0:parts, col:col + 1]

        import itertools
        _psc = itertools.count()

        def layer(name, rhs_per_kt, tag):
            """Emit the matmuls of one layer; returns list of psum tiles."""
            outs = []
            for mt, kts in enumerate(_WMETA[name]):
                width = kts[0][1]
                p = psp.tile([width, BC], f32, tag="ps", name=f"ps_{name}_{mt}_{next(_psc)}")
                for kt in range(len(kts)):
                    nc.tensor.matmul(
                        p[:], wap(name, mt, kt), rhs_per_kt[kt],
                        start=(kt == 0), stop=(kt == len(kts) - 1),
                    )
                outs.append(p)
            return outs

        def act_relu(dst, psum, bias_ap):
            nc.scalar.activation(dst, psum[:], AF.Relu, bias=bias_ap)

        def act_copy(dst, psum, bias_ap):
            nc.scalar.activation(dst, psum[:], AF.Identity, bias=bias_ap)

        def dve_relu(dst, psum, bias_ap):
            nc.vector.tensor_scalar(dst, psum[:], bias_ap, 0.0, ALU.add, ALU.max)

        # ---- rewards are staged in SBUF and flushed CH rows per DMA (keeps
        # the per-queue dynamic-DMA count low; >512 DMAs on one HWDGE queue
        # kills the exec unit)
        CH = 16
        rst = {"tile": None, "start": 0, "cnt": 0}

        def r_slot():
            if rst["tile"] is None:
                rst["tile"] = rp.tile([1, CH * BC], f32, tag="rch",
                                      name=f"rch_{rst['start']}")
            c = rst["cnt"]
            return rst["tile"][0:1, c * BC:(c + 1) * BC]

        def r_flush():
            if rst["cnt"]:
                s0, cnt = rst["start"], rst["cnt"]
                dst = rews_out[s0:s0 + cnt, :].flatten().unsqueeze(0)
                nc.scalar.dma_start(dst, rst["tile"][0:1, 0:cnt * BC])
                rst["tile"] = None
                rst["start"] = s0 + cnt
                rst["cnt"] = 0

        def r_advance():
            rst["cnt"] += 1
            if rst["cnt"] == CH:
                r_flush()

        # ---- boot
        xa_cur = xap.tile([128, BC], f32r, tag="xa")
        xf_cur = xfp.tile([D, BC], f32, tag="xf")
        nc.sync.dma_start(xa_cur[0:D, :], x0r_in[:])
        nc.sync.dma_start(xf_cur[:], x0f_in[:])

        for s in range(NSTEP):
            dwt = dwp.tile([D, BC], f32, tag="dw")
            nc.sync.dma_start(dwt[:], dw_in[s])

            # -- alpha L1 (input x only; t folded into bias) + diffusion L1
            pA1 = layer("A1", [xa_cur[0:D, :]], "ps")
            pS1 = layer("S1", [xa_cur[0:D, :]], "ps")
            h1a = [hp.tile([128, BC], f32r, tag=f"h1a{m}", name=f"h1a{m}_{s}") for m in range(2)]
            h1s = [hp.tile([128, BC], f32r, tag=f"h1s{m}", name=f"h1s{m}_{s}") for m in range(2)]
            for m in range(2):
                act_relu(h1a[m][:], pA1[m], bias("ba1", m, step=s))
                dve_relu(h1s[m][:], pS1[m], bias("bs1", m, step=s))

            # -- L2
            pA2 = layer("A2", [h1a[0][:], h1a[1][:]], "ps")
            pS2 = layer("S2", [h1s[0][:], h1s[1][:]], "ps")
            h2a = [hp.tile([128, BC], f32r, tag=f"h2a{m}", name=f"h2a{m}_{s}") for m in range(2)]
            h2s = [hp.tile([128, BC], f32r, tag=f"h2s{m}", name=f"h2s{m}_{s}") for m in range(2)]
            for m in range(2):
                act_relu(h2a[m][:], pA2[m], bias("ba2", m))
                act_relu(h2s[m][:], pS2[m], bias("bs2", m))

            # -- alpha L3 -> a written straight into xa[64:128] (f32r)
            pA3 = layer("A3", [h2a[0][:], h2a[1][:]], "ps")
            act_copy(xa_cur[D:2 * D, :], pA3[0], bias("ba3", 0, parts=D))
            nc.scalar.dma_start(acts_out[s], xa_cur[D:2 * D, :])

            # -- diffusion L3 -> w = (sig + b_s3) * dW   (psum read on DVE)
            pS3 = layer("S3", [h2s[0][:], h2s[1][:]], "ps")
            wt = uwp.tile([D, BC], f32, tag="w")
            nc.vector.scalar_tensor_tensor(
                wt[:], pS3[0][:], bias("bs3", 0, parts=D), dwt[:],
                ALU.add, ALU.mult,
            )

            # -- drift FFN (needs a, i.e. full xa)
            pD1 = layer("D1", [xa_cur[:, :]], "ps")
            h1d = [hp.tile([128, BC], f32r, tag=f"h1d{m}", name=f"h1d{m}_{s}") for m in range(2)]
            for m in range(2):
                dve_relu(h1d[m][:], pD1[m], bias("bd1", m, step=s))
            pD2 = layer("D2", [h1d[0][:], h1d[1][:]], "ps")
            h2d = [hp.tile([128, BC], f32r, tag=f"h2d{m}", name=f"h2d{m}_{s}") for m in range(2)]
            for m in range(2):
                dve_relu(h2d[m][:], pD2[m], bias("bd2", m))
            pD3 = layer("D3", [h2d[0][:], h2d[1][:]], "ps")

            # -- u = mu * dt + x_old  (exact fp32 state); close the
            # recurrence immediately: x_new as two parallel TTs (f32 exact
            # state + f32r matmul copy), then stream x out.
            ut = uwp.tile([D, BC], f32, tag="u")
            nc.vector.scalar_tensor_tensor(
                ut[:], pD3[0][:], float(dts[s]), xf_cur[:], ALU.mult, ALU.add,
            )
            xf_new = xfp.tile([D, BC], f32, tag="xf")
            xa_next = xap.tile([128, BC], f32r, tag="xa")
            nc.vector.tensor_tensor(xa_next[0:D, :], ut[:], wt[:], ALU.add)
            nc.vector.tensor_tensor(xf_new[:], ut[:], wt[:], ALU.add)
            nc.sync.dma_start(xs_out[s], xf_new[:])

            # -- running FFN (off the recurrence)
            pR1 = layer("R1", [xa_cur[:, :]], "ps")
            h1r = [hp.tile([128, BC], f32r, tag=f"h1r{m}", name=f"h1r{m}_{s}") for m in range(2)]
            for m in range(2):
                act_relu(h1r[m][:], pR1[m], bias("br1", m, step=s))
            pR2 = layer("R2", [h1r[0][:], h1r[1][:]], "ps")
            h2r = [hp.tile([128, BC], f32r, tag=f"h2r{m}", name=f"h2r{m}_{s}") for m in range(2)]
            for m in range(2):
                act_relu(h2r[m][:], pR2[m], bias("br2", m))
            pR3 = layer("R3", [h2r[0][:], h2r[1][:]], "ps")
            act_copy(r_slot(), pR3[0], bias("br3", 0, parts=1))
            r_advance()

            xa_cur, xf_cur = xa_next, xf_new

        # ---- final reward FFN on x_last
        pF1 = layer("F1", [xa_cur[0:D, :]], "ps")
        h1f = [hp.tile([128, BC], f32r, tag=f"h1a{m}", name=f"h1f{m}") for m in range(2)]
        for m in range(2):
            act_relu(h1f[m][:], pF1[m], bias("bf1", m))
        pF2 = layer("F2", [h1f[0][:], h1f[1][:]], "ps")
        h2f = [hp.tile([128, BC], f32r, tag=f"h2a{m}", name=f"h2f{m}") for m in range(2)]
        for m in range(2):
            act_relu(h2f[m][:], pF2[m], bias("bf2", m))
        pF3 = layer("F3", [h2f[0][:], h2f[1][:]], "ps")
        act_copy(r_slot(), pF3[0], bias("bf3", 0, parts=1))
        rst["cnt"] += 1
        r_flush()

    nc.finalize()
    return nc


# ------------------------------------------------------------------- host --

def _np32(a):
    return np.asarray(a, dtype=np.float32)


def _pack_params(ts, alpha_params, drift_params, diffusion_params,
                 running_params, final_params):
    """Pack weights + per-step bias tables into the two constant arrays."""
    (Wa1, ba1), (Wa2, ba2), (Wa3, ba3) = [( _np32(w), _np32(b)) for w, b in alpha_params]
    (Wd1, bd1), (Wd2, bd2), (Wd3, bd3) = [( _np32(w), _np32(b)) for w, b in drift_params]
    (Ws1, bs1), (Ws2, bs2), (Ws3, bs3) = [( _np32(w), _np32(b)) for w, b in diffusion_params]
    (Wr1, br1), (Wr2, br2), (Wr3, br3) = [( _np32(w), _np32(b)) for w, b in running_params]
    (Wf1, bf1), (Wf2, bf2), (Wf3, bf3) = [( _np32(w), _np32(b)) for w, b in final_params]

    mats = {
        "A1": Wa1[1:, :], "A2": Wa2, "A3": Wa3,
        "R1": Wr1, "R2": Wr2, "R3": Wr3,
        "D1": Wd1, "D2": Wd2, "D3": Wd3,
        "S1": Ws1, "S2": Ws2, "S3": Ws3,
        "F1": Wf1, "F2": Wf2, "F3": Wf3,
    }
    wr = np.zeros((128, NW), np.float32)
    for name, blocks_m in _WMETA.items():
        Wm = mats[name]
        for blocks_k in blocks_m:
            for (col, width, rows, k0, m0) in blocks_k:
                wr[0:rows, col:col + width] = Wm[k0:k0 + rows, m0:m0 + width]

    ts64 = np.asarray(ts, np.float64)
    bmu = bd3.astype(np.float64)                      # (D,) drift L3 bias
    # per-step L1 bias tables: base + t_s * vec   (vec folds the x' shift and,
    # for alpha, the time input column)
    va = Wa1[0, :].astype(np.float64) + Wa1[1:, :].astype(np.float64).T @ bmu
    vr = Wr1[:D, :].astype(np.float64).T @ bmu
    vd = Wd1[:D, :].astype(np.float64).T @ bmu
    vs = Ws1.astype(np.float64).T @ bmu

    cb = np.zeros((128, NB), np.float32)

    def fill_table(key, base, vec):
        tab = base.astype(np.float64)[None, :] + ts64[:NSTEP, None] * vec[None, :]
        tab = tab.astype(np.float32)                  # (NSTEP, 256)
        for k in range(2):
            cols = slice(_CB[key] + k * NSTEP, _CB[key] + (k + 1) * NSTEP)
            cb[:, cols] = tab[:, k * 128:(k + 1) * 128].T

    fill_table("ba1", ba1, va)
    fill_table("br1", br1, vr)
    fill_table("bd1", bd1, vd)
    fill_table("bs1", bs1, vs)

    def fill_static(key, vec):
        v = _np32(vec)
        for k in range(0, len(v), 128):
            w = min(128, len(v) - k)
            cb[0:w, _CB[key] + k // 128] = v[k:k + w]

    fill_static("ba2", ba2)
    fill_static("ba3", ba3)
    fill_static("br2", br2)
    fill_static("br3", br3)
    fill_static("bd2", bd2)
    fill_static("bs2", bs2)
    fill_static("bs3", bs3)
    bf1c = (bf1.astype(np.float64) + Wf1.astype(np.float64).T @ (ts64[NSTEP] * bmu))
    fill_static("bf1", bf1c.astype(np.float32))
    fill_static("bf2", bf2)
    fill_static("bf3", bf3)

    # trajectory correction C_s = t_s * b_mu (add back on host); (L, D) f64
    C = ts64[:, None] * bmu[None, :]
    return wr, cb, C


def kernel(ts, x0, brownian_increments, alpha_params, drift_params,
           diffusion_params, running_params, final_params):
    ts = _np32(ts)
    x0 = _np32(x0)
    brown = _np32(brownian_increments)
    assert x0.shape == (B, D) and brown.shape == (B, L - 1, D)

    h = ts[1:] - ts[:-1]                             # fp32, as the reference
    dts = [float(v) for v in h[:NSTEP]]

    key = ("prog", NSTEP)
    if key not in _PROG_CACHE:
        _PROG_CACHE[key] = _build_program(dts)
    nc = _PROG_CACHE[key]

    wr, cb, C = _pack_params(ts, alpha_params, drift_params, diffusion_params,
                             running_params, final_params)

    in_maps = []
    for c in range(NCORES):
        sl = slice(c * BC, (c + 1) * BC)
        x0t = np.ascontiguousarray(x0[sl].T)          # (D, BC)
        dwt = np.ascontiguousarray(brown[sl, :NSTEP].transpose(1, 2, 0))  # (NSTEP, D, BC)
        in_maps.append({
            "x0r": x0t, "x0f": x0t, "dw": dwt, "wr": wr, "cb": cb,
        })

    trace = bool(os.environ.get("KSDE_TRACE"))
    res = run_bass_kernel_spmd(nc, in_maps, core_ids=list(range(NCORES)),
                               trace=trace)
    kernel.last_results = res

    x_full = np.empty((B, L, D), np.float32)
    a_full = np.empty((B, L - 1, D), np.float32)
    r_full = np.zeros((B, L, 1), np.float32)
    x_full[:, 0] = x0
    corr = C.astype(np.float32)                       # (L, D)
    for c in range(NCORES):
        sl = slice(c * BC, (c + 1) * BC)
        r = res.results[c]
        xs = r["xs"].transpose(2, 0, 1)               # (BC, NSTEP, D)
        x_full[sl, 1:NSTEP + 1] = xs + corr[1:NSTEP + 1][None, :, :]
        a_full[sl, :NSTEP] = r["acts"].transpose(2, 0, 1)
        r_full[sl, :NSTEP + 1, 0] = r["rews"].T
    return x_full, brown, a_full, r_full


# revision 10
# speedup vs baseline: 1.0537x; 1.0537x over previous
"""Controlled neural-SDE Euler-Maruyama kernel for 8 Trainium2 NeuronCores.

Strategy (data-parallel over batch, per sharding hint):
  - batch 4096 -> 8 cores x 512. Each core runs the full 199-step time loop.
  - Feature-major layout on chip: activations are (features, batch) so every
    FFN layer is matmul(out_psum, lhsT=W_block, rhs=act) with the batch (512)
    as the moving/free dimension. Weights are packed once into one SBUF tile.
  - Matmuls run in float32r (TF32-class, 1 cycle/row at free dim 512 vs 4 for
    fp32). The x state is kept in exact fp32 in a separate tile; a rounded
    float32r copy feeds the matmuls.
  - The time/bias terms are folded into per-step L1 bias tables computed on
    the host:
      * alpha input concat([t, x]) == matmul over x + per-step bias
        t_s * Wa1[0,:].
      * drift L3 bias b_mu is folded into the trajectory reparametrization
        x'_s = x_s - t_s * b_mu; all four L1 layers get the per-step bias
        correction W1x.T @ (t_s * b_mu), and the host adds t_s * b_mu back to
        the x output. (With the reference's zero-init biases this is a no-op.)
  - Per step: 32 matmuls, 10 scalar-engine activations (bias+relu psum->sbuf),
    12 vector-engine ops (bias+relu copies for drift/diffusion, then the
    fused update u=(mu*dt+x), w=(sig+b)*dW, x_new=u+w, and the f32r copy).
  - rewards row, action tile and x tile are streamed out per step via HWDGE.
"""
import os
import sys

sys.path.insert(0, "/opt/trn_rl_repo")

import numpy as np
from contextlib import ExitStack

import concourse.bass as bass
import concourse.bacc as bacc
import concourse.mybir as mybir
from concourse import tile
from concourse.bass_utils import run_bass_kernel_spmd

# Optional on-disk NEFF cache for the dev loop (harmless if unset).
if os.environ.get("KSDE_NEFF_CACHE"):
    import hashlib
    import shutil
    import concourse.bass2jax as _b2j

    _CACHE_DIR = os.environ["KSDE_NEFF_CACHE"]
    os.makedirs(_CACHE_DIR, exist_ok=True)
    _orig_compile = _b2j.compile_bir_kernel

    def _cached_compile(bir_json, tmpdir, neff_name="file.neff"):
        key = hashlib.sha256(bir_json).hexdigest()
        cached = os.path.join(_CACHE_DIR, key + ".neff")
        if os.path.exists(cached):
            dst = os.path.join(tmpdir, neff_name)
            shutil.copy(cached, dst)
            return dst
        out = _orig_compile(bir_json, tmpdir, neff_name)
        shutil.copy(out, cached)
        return out

    _b2j.compile_bir_kernel = _cached_compile

B, L, D = 4096, 200, 64
HID = 256
NCORES = 8
BC = B // NCORES
NSTEP = int(os.environ.get("KSDE_NSTEP", L - 1))

f32 = mybir.dt.float32
f32r = mybir.dt.float32r
f16 = mybir.dt.float16
AF = mybir.ActivationFunctionType
ALU = mybir.AluOpType

# ---------------------------------------------------------------- layouts --

_MATS = {
    # name: (K_rows, M_cols)   (lhsT layout: W[k, m] with k on partitions)
    "A1": (D, HID), "A2": (HID, HID), "A3": (HID, D),
    "R1": (2 * D, HID), "R2": (HID, HID), "R3": (HID, 1),
    "D1": (2 * D, HID), "D2": (HID, HID), "D3": (HID, D),
    "S1": (D, HID), "S2": (HID, HID), "S3": (HID, D),
    "F1": (D, HID), "F2": (HID, HID), "F3": (HID, 1),
}


def _build_wmeta():
    """Column layout of the packed weights tile: wmeta[name][mt][kt] ->
    (col, width, rows)."""
    wmeta = {}
    col = 0
    for name, (K, M) in _MATS.items():
        blocks_m = []
        for m0 in range(0, M, 128):
            width = min(128, M - m0)
            blocks_k = []
            for k0 in range(0, K, 128):
                rows = min(128, K - k0)
                blocks_k.append((col, width, rows, k0, m0))
                col += width
            blocks_m.append(blocks_k)
        wmeta[name] = blocks_m
    return wmeta, col


_WMETA, NW = _build_wmeta()

# constants tile columns
_CB = {}
_cb_col = 0
for _name, _width in [
    ("ba1", 2 * NSTEP), ("br1", 2 * NSTEP), ("bd1", 2 * NSTEP),
    ("bs1", 2 * NSTEP),
    ("ba2", 2), ("ba3", 1), ("br2", 2), ("br3", 1), ("bd2", 2),
    ("bs2", 2), ("bs3", 1), ("bf1", 2), ("bf2", 2), ("bf3", 1),
]:
    _CB[_name] = _cb_col
    _cb_col += _width
NB = _cb_col

# ---------------------------------------------------------------- program --

_PROG_CACHE = {}


def _build_program(dts):
    """Build (and schedule) the per-core Bass program. dts: python floats of
    the 199 fp32 time deltas (compile-time immediates)."""
    nc = bacc.Bacc("TRN2", target_bir_lowering=False, debug=False)

    x0r_in = nc.declare_dram_parameter("x0r", [D, BC], f16, isOutput=False)
    x0f_in = nc.declare_dram_parameter("x0f", [D, BC], f32, isOutput=False)
    dw_in = nc.declare_dram_parameter("dw", [NSTEP, D, BC], f32, isOutput=False)
    wr_in = nc.declare_dram_parameter("wr", [128, NW], f16, isOutput=False)
    cb_in = nc.declare_dram_parameter("cb", [128, NB], f32, isOutput=False)

    xs_out = nc.declare_dram_parameter("xs", [NSTEP, D, BC], f32, isOutput=True)
    acts_out = nc.declare_dram_parameter("acts", [NSTEP, D, BC], f16, isOutput=True)
    rews_out = nc.declare_dram_parameter("rews", [NSTEP + 1, BC], f32, isOutput=True)

    with ExitStack() as ctx:
        tc = ctx.enter_context(tile.TileContext(nc))
        wp = ctx.enter_context(tc.tile_pool(name="wp", bufs=1))
        xap = ctx.enter_context(tc.tile_pool(name="xap", bufs=3))
        xfp = ctx.enter_context(tc.tile_pool(name="xfp", bufs=3))
        dwp = ctx.enter_context(tc.tile_pool(name="dwp", bufs=4))
        hp = ctx.enter_context(tc.tile_pool(name="hp", bufs=2))
        uwp = ctx.enter_context(tc.tile_pool(name="uwp", bufs=2))
        rp = ctx.enter_context(tc.tile_pool(name="rp", bufs=2))
        psp = ctx.enter_context(tc.tile_pool(name="psp", bufs=8, space="PSUM"))

        wrt = wp.tile([128, NW], f16)
        cbt = wp.tile([128, NB], f32)
        nc.sync.dma_start(wrt[:], wr_in[:])
        nc.sync.dma_start(cbt[:], cb_in[:])

        def wap(name, mt, kt):
            col, width, rows, _, _ = _WMETA[name][mt][kt]
            return wrt[0:rows, col:col + width]

        def bias(name, k, parts=128, step=None):
            col = _CB[name] + (k * NSTEP + step if step is not None else k)
            return cbt[0:parts, col:col + 1]

        import itertools
        _psc = itertools.count()

        def layer(name, rhs_per_kt, tag):
            """Emit the matmuls of one layer; returns list of psum tiles."""
            outs = []
            for mt, kts in enumerate(_WMETA[name]):
                width = kts[0][1]
                p = psp.tile([width, BC], f32, tag="ps", name=f"ps_{name}_{mt}_{next(_psc)}")
                for kt in range(len(kts)):
                    nc.tensor.matmul(
                        p[:], wap(name, mt, kt), rhs_per_kt[kt],
                        start=(kt == 0), stop=(kt == len(kts) - 1),
                    )
                outs.append(p)
            return outs

        def act_relu(dst, psum, bias_ap):
            nc.scalar.activation(dst, psum[:], AF.Relu, bias=bias_ap)

        def act_copy(dst, psum, bias_ap):
            nc.scalar.activation(dst, psum[:], AF.Identity, bias=bias_ap)

        def dve_relu(dst, psum, bias_ap):
            nc.vector.tensor_scalar(dst, psum[:], bias_ap, 0.0, ALU.add, ALU.max)

        # ---- rewards are staged in SBUF and flushed CH rows per DMA (keeps
        # the per-queue dynamic-DMA count low; >512 DMAs on one HWDGE queue
        # kills the exec unit)
        CH = 16
        rst = {"tile": None, "start": 0, "cnt": 0}

        def r_slot():
            if rst["tile"] is None:
                rst["tile"] = rp.tile([1, CH * BC], f32, tag="rch",
                                      name=f"rch_{rst['start']}")
            c = rst["cnt"]
            return rst["tile"][0:1, c * BC:(c + 1) * BC]

        def r_flush():
            if rst["cnt"]:
                s0, cnt = rst["start"], rst["cnt"]
                dst = rews_out[s0:s0 + cnt, :].flatten().unsqueeze(0)
                nc.scalar.dma_start(dst, rst["tile"][0:1, 0:cnt * BC])
                rst["tile"] = None
                rst["start"] = s0 + cnt
                rst["cnt"] = 0

        def r_advance():
            rst["cnt"] += 1
            if rst["cnt"] == CH:
                r_flush()

        # ---- boot
        xa_cur = xap.tile([128, BC], f16, tag="xa")
        xf_cur = xfp.tile([D, BC], f32, tag="xf")
        nc.sync.dma_start(xa_cur[0:D, :], x0r_in[:])
        nc.sync.dma_start(xf_cur[:], x0f_in[:])

        for s in range(NSTEP):
            dwt = dwp.tile([D, BC], f32, tag="dw")
            nc.sync.dma_start(dwt[:], dw_in[s])

            # -- alpha L1 (input x only; t folded into bias) + diffusion L1
            pA1 = layer("A1", [xa_cur[0:D, :]], "ps")
            pS1 = layer("S1", [xa_cur[0:D, :]], "ps")
            h1a = [hp.tile([128, BC], f16, tag=f"h1a{m}", name=f"h1a{m}_{s}") for m in range(2)]
            h1s = [hp.tile([128, BC], f16, tag=f"h1s{m}", name=f"h1s{m}_{s}") for m in range(2)]
            for m in range(2):
                act_relu(h1a[m][:], pA1[m], bias("ba1", m, step=s))
                dve_relu(h1s[m][:], pS1[m], bias("bs1", m, step=s))

            # -- L2
            pA2 = layer("A2", [h1a[0][:], h1a[1][:]], "ps")
            pS2 = layer("S2", [h1s[0][:], h1s[1][:]], "ps")
            h2a = [hp.tile([128, BC], f16, tag=f"h2a{m}", name=f"h2a{m}_{s}") for m in range(2)]
            h2s = [hp.tile([128, BC], f16, tag=f"h2s{m}", name=f"h2s{m}_{s}") for m in range(2)]
            for m in range(2):
                act_relu(h2a[m][:], pA2[m], bias("ba2", m))
                act_relu(h2s[m][:], pS2[m], bias("bs2", m))

            # -- alpha L3 -> a written straight into xa[64:128] (f32r)
            pA3 = layer("A3", [h2a[0][:], h2a[1][:]], "ps")
            act_copy(xa_cur[D:2 * D, :], pA3[0], bias("ba3", 0, parts=D))
            nc.scalar.dma_start(acts_out[s], xa_cur[D:2 * D, :])

            # -- diffusion L3 -> w = (sig + b_s3) * dW   (psum read on DVE)
            pS3 = layer("S3", [h2s[0][:], h2s[1][:]], "ps")
            wt = uwp.tile([D, BC], f32, tag="w")
            nc.vector.scalar_tensor_tensor(
                wt[:], pS3[0][:], bias("bs3", 0, parts=D), dwt[:],
                ALU.add, ALU.mult,
            )

            # -- drift FFN (needs a, i.e. full xa)
            pD1 = layer("D1", [xa_cur[:, :]], "ps")
            h1d = [hp.tile([128, BC], f16, tag=f"h1d{m}", name=f"h1d{m}_{s}") for m in range(2)]
            for m in range(2):
                dve_relu(h1d[m][:], pD1[m], bias("bd1", m, step=s))
            pD2 = layer("D2", [h1d[0][:], h1d[1][:]], "ps")
            h2d = [hp.tile([128, BC], f16, tag=f"h2d{m}", name=f"h2d{m}_{s}") for m in range(2)]
            for m in range(2):
                dve_relu(h2d[m][:], pD2[m], bias("bd2", m))
            pD3 = layer("D3", [h2d[0][:], h2d[1][:]], "ps")

            # -- u = mu * dt + x_old  (exact fp32 state); close the
            # recurrence immediately: x_new as two parallel TTs (f32 exact
            # state + f32r matmul copy), then stream x out.
            ut = uwp.tile([D, BC], f32, tag="u")
            nc.vector.scalar_tensor_tensor(
                ut[:], pD3[0][:], float(dts[s]), xf_cur[:], ALU.mult, ALU.add,
            )
            xf_new = xfp.tile([D, BC], f32, tag="xf")
            xa_next = xap.tile([128, BC], f16, tag="xa")
            nc.vector.tensor_tensor(xa_next[0:D, :], ut[:], wt[:], ALU.add)
            nc.vector.tensor_tensor(xf_new[:], ut[:], wt[:], ALU.add)
            nc.sync.dma_start(xs_out[s], xf_new[:])

            # -- running FFN (off the recurrence)
            pR1 = layer("R1", [xa_cur[:, :]], "ps")
            h1r = [hp.tile([128, BC], f16, tag=f"h1r{m}", name=f"h1r{m}_{s}") for m in range(2)]
            for m in range(2):
                act_relu(h1r[m][:], pR1[m], bias("br1", m, step=s))
            pR2 = layer("R2", [h1r[0][:], h1r[1][:]], "ps")
            h2r = [hp.tile([128, BC], f16, tag=f"h2r{m}", name=f"h2r{m}_{s}") for m in range(2)]
            for m in range(2):
                act_relu(h2r[m][:], pR2[m], bias("br2", m))
            pR3 = layer("R3", [h2r[0][:], h2r[1][:]], "ps")
            act_copy(r_slot(), pR3[0], bias("br3", 0, parts=1))
            r_advance()

            xa_cur, xf_cur = xa_next, xf_new

        # ---- final reward FFN on x_last
        pF1 = layer("F1", [xa_cur[0:D, :]], "ps")
        h1f = [hp.tile([128, BC], f16, tag=f"h1a{m}", name=f"h1f{m}") for m in range(2)]
        for m in range(2):
            act_relu(h1f[m][:], pF1[m], bias("bf1", m))
        pF2 = layer("F2", [h1f[0][:], h1f[1][:]], "ps")
        h2f = [hp.tile([128, BC], f16, tag=f"h2a{m}", name=f"h2f{m}") for m in range(2)]
        for m in range(2):
            act_relu(h2f[m][:], pF2[m], bias("bf2", m))
        pF3 = layer("F3", [h2f[0][:], h2f[1][:]], "ps")
        act_copy(r_slot(), pF3[0], bias("bf3", 0, parts=1))
        rst["cnt"] += 1
        r_flush()

    nc.finalize()
    return nc


# ------------------------------------------------------------------- host --

def _np32(a):
    return np.asarray(a, dtype=np.float32)


def _pack_params(ts, alpha_params, drift_params, diffusion_params,
                 running_params, final_params):
    """Pack weights + per-step bias tables into the two constant arrays."""
    (Wa1, ba1), (Wa2, ba2), (Wa3, ba3) = [( _np32(w), _np32(b)) for w, b in alpha_params]
    (Wd1, bd1), (Wd2, bd2), (Wd3, bd3) = [( _np32(w), _np32(b)) for w, b in drift_params]
    (Ws1, bs1), (Ws2, bs2), (Ws3, bs3) = [( _np32(w), _np32(b)) for w, b in diffusion_params]
    (Wr1, br1), (Wr2, br2), (Wr3, br3) = [( _np32(w), _np32(b)) for w, b in running_params]
    (Wf1, bf1), (Wf2, bf2), (Wf3, bf3) = [( _np32(w), _np32(b)) for w, b in final_params]

    mats = {
        "A1": Wa1[1:, :], "A2": Wa2, "A3": Wa3,
        "R1": Wr1, "R2": Wr2, "R3": Wr3,
        "D1": Wd1, "D2": Wd2, "D3": Wd3,
        "S1": Ws1, "S2": Ws2, "S3": Ws3,
        "F1": Wf1, "F2": Wf2, "F3": Wf3,
    }
    wr = np.zeros((128, NW), np.float16)
    for name, blocks_m in _WMETA.items():
        Wm = mats[name]
        for blocks_k in blocks_m:
            for (col, width, rows, k0, m0) in blocks_k:
                wr[0:rows, col:col + width] = Wm[k0:k0 + rows, m0:m0 + width]

    ts64 = np.asarray(ts, np.float64)
    bmu = bd3.astype(np.float64)                      # (D,) drift L3 bias
    # per-step L1 bias tables: base + t_s * vec   (vec folds the x' shift and,
    # for alpha, the time input column)
    va = Wa1[0, :].astype(np.float64) + Wa1[1:, :].astype(np.float64).T @ bmu
    vr = Wr1[:D, :].astype(np.float64).T @ bmu
    vd = Wd1[:D, :].astype(np.float64).T @ bmu
    vs = Ws1.astype(np.float64).T @ bmu

    cb = np.zeros((128, NB), np.float32)

    def fill_table(key, base, vec):
        tab = base.astype(np.float64)[None, :] + ts64[:NSTEP, None] * vec[None, :]
        tab = tab.astype(np.float32)                  # (NSTEP, 256)
        for k in range(2):
            cols = slice(_CB[key] + k * NSTEP, _CB[key] + (k + 1) * NSTEP)
            cb[:, cols] = tab[:, k * 128:(k + 1) * 128].T

    fill_table("ba1", ba1, va)
    fill_table("br1", br1, vr)
    fill_table("bd1", bd1, vd)
    fill_table("bs1", bs1, vs)

    def fill_static(key, vec):
        v = _np32(vec)
        for k in range(0, len(v), 128):
            w = min(128, len(v) - k)
            cb[0:w, _CB[key] + k // 128] = v[k:k + w]

    fill_static("ba2", ba2)
    fill_static("ba3", ba3)
    fill_static("br2", br2)
    fill_static("br3", br3)
    fill_static("bd2", bd2)
    fill_static("bs2", bs2)
    fill_static("bs3", bs3)
    bf1c = (bf1.astype(np.float64) + Wf1.astype(np.float64).T @ (ts64[NSTEP] * bmu))
    fill_static("bf1", bf1c.astype(np.float32))
    fill_static("bf2", bf2)
    fill_static("bf3", bf3)

    # trajectory correction C_s = t_s * b_mu (add back on host); (L, D) f64
    C = ts64[:, None] * bmu[None, :]
    return wr, cb, C


def kernel(ts, x0, brownian_increments, alpha_params, drift_params,
           diffusion_params, running_params, final_params):
    ts = _np32(ts)
    x0 = _np32(x0)
    brown = _np32(brownian_increments)
    assert x0.shape == (B, D) and brown.shape == (B, L - 1, D)

    h = ts[1:] - ts[:-1]                             # fp32, as the reference
    dts = [float(v) for v in h[:NSTEP]]

    key = ("prog", NSTEP)
    if key not in _PROG_CACHE:
        _PROG_CACHE[key] = _build_program(dts)
    nc = _PROG_CACHE[key]

    wr, cb, C = _pack_params(ts, alpha_params, drift_params, diffusion_params,
                             running_params, final_params)

    in_maps = []
    for c in range(NCORES):
        sl = slice(c * BC, (c + 1) * BC)
        x0t = np.ascontiguousarray(x0[sl].T)          # (D, BC)
        x0h = x0t.astype(np.float16)
        dwt = np.ascontiguousarray(brown[sl, :NSTEP].transpose(1, 2, 0))  # (NSTEP, D, BC)
        in_maps.append({
            "x0r": x0h, "x0f": x0t, "dw": dwt, "wr": wr, "cb": cb,
        })

    trace = bool(os.environ.get("KSDE_TRACE"))
    res = run_bass_kernel_spmd(nc, in_maps, core_ids=list(range(NCORES)),
                               trace=trace)
    kernel.last_results = res

    x_full = np.empty((B, L, D), np.float32)
    a_full = np.empty((B, L - 1, D), np.float32)
    r_full = np.zeros((B, L, 1), np.float32)
    x_full[:, 0] = x0
    corr = C.astype(np.float32)                       # (L, D)
    for c in range(NCORES):
        sl = slice(c * BC, (c + 1) * BC)
        r = res.results[c]
        xs = r["xs"].transpose(2, 0, 1)               # (BC, NSTEP, D)
        x_full[sl, 1:NSTEP + 1] = xs + corr[1:NSTEP + 1][None, :, :]
        a_full[sl, :NSTEP] = r["acts"].transpose(2, 0, 1).astype(np.float32)
        r_full[sl, :NSTEP + 1, 0] = r["rews"].T
    return x_full, brown, a_full, r_full
